# revision 2
# baseline (speedup 1.0000x reference)
"""Trainium2 Bass kernel for masked attention-score softmax — v5.

Baseline dataflow (DVE fused multiply+accumulate over streamed x), with
the per-example epilogue restructured OFF the inter-example critical
path:

  - eij stays in [128t, 32col] layout; tanh/exp run on ACT directly
    (no PE pre-transpose feeding ACT).
  - mask-multiply and the row sum fuse into ONE DVE stt (accum_out).
  - the cross-partition sum + broadcast is ONE PE matmul against an
    all-ones [128,128] stationary (replaces gpsimd partition_all_reduce,
    which put a slow Q7 round-trip inside DVE's in-order stream).
  - EPS folds into the ACT copy of the sum (Copy with bias).
  - the output row transpose happens at the tail (PE+ACT), and the
    output DMA issues from the ACT HWDGE ring, not the sync-engine ring,
    so x-chunk DMA issuance never queues behind an epilogue-gated store.

Per example, DVE's non-stt footprint is ~2 small ops and SP's stream
contains only loads: the x stream and the DVE dot products pipeline
straight through example and rep boundaries.
"""

import numpy as np

from contextlib import ExitStack

import concourse.bass as bass
import concourse.bass_isa as bass_isa
import concourse.tile as tile
from concourse import bacc, masks, mybir
from concourse.bass_utils import run_bass_kernel_spmd

FP32 = mybir.dt.float32
U8 = mybir.dt.uint8

N_CORES = 8
B_FULL, T, D, E = 32, 4096, 512, 512
B = B_FULL // N_CORES  # 4 examples per core
P = 128
EPS = 1e-7

TBLK = T // P           # 32 t-blocks of 128 rows per example
CHUNK = 8               # t-blocks per x DMA (128 x 8 x 512 f32 = 2 MiB)
NCHUNK = TBLK // CHUNK  # 4 chunks per example
XBUFS = 6


def build_program(reps: int = 1):
    nc = bacc.Bacc(
        "TRN2",
        target_bir_lowering=False,
        debug=False,
        num_devices=N_CORES,
    )

    x_ap = nc.dram_tensor("x", [B, T, D], FP32, kind="ExternalInput").ap()
    y_ap = nc.dram_tensor("y", [B, E], FP32, kind="ExternalInput").ap()
    w_ap = nc.dram_tensor("W", [D, E], FP32, kind="ExternalInput").ap()
    b_ap = nc.dram_tensor("b", [1, 1], FP32, kind="ExternalInput").ap()
    m_ap = nc.dram_tensor("mask", [B, T], U8, kind="ExternalInput").ap()
    o_ap = nc.dram_tensor("out", [B, T], FP32, kind="ExternalOutput").ap()

    with tile.TileContext(nc) as tc, ExitStack() as ctx:
        singles = ctx.enter_context(tc.tile_pool(name="singles", bufs=1))
        xpool = ctx.enter_context(tc.tile_pool(name="xpool", bufs=XBUFS))
        eijpool = ctx.enter_context(tc.tile_pool(name="eij", bufs=2))
        small = ctx.enter_context(tc.tile_pool(name="small", bufs=2))
        ps_big = ctx.enter_context(tc.tile_pool(name="ps_big", bufs=2, space="PSUM"))
        ps_small = ctx.enter_context(
            tc.tile_pool(name="ps_small", bufs=2, space="PSUM")
        )
        ps_et = ctx.enter_context(tc.tile_pool(name="ps_et", bufs=2, space="PSUM"))

        def body():
            # ---- constants ----
            identity = singles.tile([P, P], FP32)
            masks.make_identity(nc, identity[:])
            ones_row = singles.tile([1, P], FP32)
            nc.vector.memset(ones_row[:], 1.0)
            ones_sq = singles.tile([P, P], FP32)
            nc.vector.memset(ones_sq[:], 1.0)
            dummy = singles.tile([P, 1], FP32)

            # ---- W^T on chip ----
            w_sb = singles.tile([P, D // P, E], FP32)
            nc.sync.dma_start(w_sb[:], w_ap.rearrange("(dc p) e -> p dc e", p=P))
            wt_sb = singles.tile([P, E // P, D], FP32)
            for ec in range(E // P):
                wt_ps = ps_big.tile([P, D], FP32, tag="big")
                for dc in range(D // P):
                    nc.tensor.transpose(
                        wt_ps[:, dc * P : (dc + 1) * P],
                        w_sb[:, dc, ec * P : (ec + 1) * P],
                        identity[:],
                    )
                nc.scalar.copy(wt_sb[:, ec, :], wt_ps[:])

            # ---- y^T on chip ----
            y_sb = singles.tile([B, E], FP32)
            nc.sync.dma_start(y_sb[:], y_ap)
            yt_ps = ps_small.tile([P, E // P, B], FP32, tag="small")
            for ec in range(E // P):
                nc.tensor.transpose(
                    yt_ps[:, ec, :],
                    y_sb[:, ec * P : (ec + 1) * P],
                    identity[:B, :B],
                )
            yt_sb = singles.tile([P, E // P, B], FP32)
            nc.scalar.copy(yt_sb[:], yt_ps[:])

            # ---- yp = y @ W.T  -> [B, D] ----
            yp_ps = ps_small.tile([B, D], FP32, tag="small")
            for ec in range(E // P):
                nc.tensor.matmul(
                    yp_ps[:],
                    yt_sb[:, ec, :],
                    wt_sb[:, ec, :],
                    start=(ec == 0),
                    stop=(ec == E // P - 1),
                )
            yp_sb = singles.tile([B, D], FP32)
            nc.scalar.copy(yp_sb[:], yp_ps[:])

            # ---- broadcast yp rows across partitions ----
            sel = singles.tile([B, B, P], FP32)
            nc.gpsimd.memset(sel[:], 0.0)
            nc.gpsimd.affine_select(
                out=sel[:],
                in_=sel[:],
                compare_op=mybir.AluOpType.not_equal,
                fill=1.0,
                base=0,
                pattern=[[-1, B], [0, P]],
                channel_multiplier=1,
            )
            yp_bcast = singles.tile([P, B, D], FP32)
            for bi in range(B):
                ypb_ps = ps_big.tile([P, D], FP32, tag="big")
                nc.tensor.matmul(
                    ypb_ps[:],
                    sel[:, bi, :],
                    yp_sb[:],
                    start=True,
                    stop=True,
                )
                nc.scalar.copy(yp_bcast[:, bi, :], ypb_ps[:])

            # ---- bias broadcast to [128, 1] ----
            b_sb = singles.tile([1, 1], FP32)
            nc.sync.dma_start(b_sb[:], b_ap)
            b_ps = ps_small.tile([P, 1], FP32, tag="small")
            nc.tensor.matmul(
                b_ps[:], ones_row[:], b_sb[:], start=True, stop=True
            )
            b_bcast = singles.tile([P, 1], FP32)
            nc.scalar.copy(b_bcast[:], b_ps[:])

            # ---- mask -> f32 in eij layout: m_t[p, bi, c] = mask[bi, c*P+p]
            m_u8 = singles.tile([TBLK, B, P], U8)
            nc.sync.dma_start(m_u8[:], m_ap.rearrange("b (c q) -> c b q", q=P))
            m_f = singles.tile([TBLK, B, P], FP32)
            nc.vector.tensor_copy(out=m_f[:], in_=m_u8[:])
            mt_ps = ps_small.tile([P, B, TBLK], FP32, tag="small")
            for bi in range(B):
                nc.tensor.transpose(
                    mt_ps[:, bi, :], m_f[:, bi, :], identity[:TBLK, :TBLK]
                )
            m_t = singles.tile([P, B, TBLK], FP32)
            nc.scalar.copy(m_t[:], mt_ps[:])

            # ---- main loop over examples ----
            for bi in range(B):
                xr = x_ap[bi].rearrange("(n p) d -> p n d", p=P)  # [128, 32, 512]
                eij = eijpool.tile([P, TBLK], FP32)
                for c in range(NCHUNK):
                    xt = xpool.tile([P, CHUNK, D], FP32)
                    nc.sync.dma_start(xt[:], xr[:, c * CHUNK : (c + 1) * CHUNK, :])
                    for j in range(CHUNK):
                        col = c * CHUNK + j
                        # out = (x * 1.0) * yp ; accum_out = sum over free dim
                        nc.vector.scalar_tensor_tensor(
                            out=dummy.broadcast_to([P, D]),
                            in0=xt[:, j, :],
                            scalar=1.0,
                            in1=yp_bcast[:, bi, :],
                            op0=mybir.AluOpType.mult,
                            op1=mybir.AluOpType.mult,
                            accum_out=eij[:, col : col + 1],
                        )

                # ---- epilogue for example bi (in [128t, 32c] layout) ----
                s = small.tile([P, TBLK], FP32, tag="s")
                nc.scalar.activation(
                    s[:],
                    eij[:],
                    mybir.ActivationFunctionType.Tanh,
                    bias=b_bcast[:],
                    scale=1.0,
                )
                ex = small.tile([P, TBLK], FP32, tag="ex")
                nc.scalar.activation(ex[:], s[:], mybir.ActivationFunctionType.Exp)
                # am = ex * mask ; colsum = row-wise sum(am)
                am = small.tile([P, TBLK], FP32, tag="am")
                colsum = small.tile([P, 1], FP32, tag="cs")
                nc.vector.scalar_tensor_tensor(
                    out=am[:],
                    in0=ex[:],
                    scalar=1.0,
                    in1=m_t[:, bi, :],
                    op0=mybir.AluOpType.mult,
                    op1=mybir.AluOpType.mult,
                    accum_out=colsum[:],
                )
                # total = sum_p colsum, broadcast to all 128 partitions
                sum_ps = ps_small.tile([P, 1], FP32, tag="sum")
                nc.tensor.matmul(
                    sum_ps[:], ones_sq[:], colsum[:], start=True, stop=True
                )
                den = small.tile([P, 1], FP32, tag="den")
                nc.scalar.activation(
                    den[:],
                    sum_ps[:],
                    mybir.ActivationFunctionType.Copy,
                    bias=EPS,
                    scale=1.0,
                )
                rcp = small.tile([P, 1], FP32, tag="rcp")
                nc.vector.reciprocal(rcp[:], den[:])
                a_sc = small.tile([P, TBLK], FP32, tag="asc")
                nc.scalar.activation(
                    a_sc[:],
                    am[:],
                    mybir.ActivationFunctionType.Copy,
                    bias=0.0,
                    scale=rcp[:],
                )
                # transpose to [32c, 128q] rows for a contiguous store
                at_ps = ps_et.tile([TBLK, P], FP32, tag="et")
                nc.tensor.transpose(at_ps[:], a_sc[:], identity[:])
                a_out = small.tile([TBLK, P], FP32, tag="aout")
                nc.scalar.copy(a_out[:], at_ps[:])
                nc.scalar.dma_start(
                    o_ap[bi].rearrange("(c q) -> c q", q=P), a_out[:]
                )

        for _ in range(reps):
            body()

    nc.compile()
    return nc


_NC_CACHE = {}


def _get_nc(reps: int = 1):
    if reps not in _NC_CACHE:
        _NC_CACHE[reps] = build_program(reps)
    return _NC_CACHE[reps]


def make_in_maps(x, y, W, b, mask):
    x = np.ascontiguousarray(x, dtype=np.float32)
    y = np.ascontiguousarray(y, dtype=np.float32)
    W = np.ascontiguousarray(W, dtype=np.float32)
    b = np.ascontiguousarray(b, dtype=np.float32).reshape(1, 1)
    mask_u8 = np.ascontiguousarray(mask).view(np.uint8)
    in_maps = []
    for i in range(N_CORES):
        sl = slice(i * B, (i + 1) * B)
        in_maps.append(
            {
                "x": x[sl],
                "y": y[sl],
                "W": W,
                "b": b,
                "mask": mask_u8[sl],
            }
        )
    return in_maps


def run(x, y, W, b, mask, trace=False, **kw):
    nc = _get_nc()
    in_maps = make_in_maps(x, y, W, b, mask)
    res = run_bass_kernel_spmd(
        nc, in_maps, core_ids=list(range(N_CORES)), trace=trace, **kw
    )
    out = np.concatenate([r["out"] for r in res.results], axis=0)
    return out, res


def kernel(x, y, W, b, mask):
    out, _ = run(x, y, W, b, mask)
    return out


# ---------------------------------------------------------------------------
# Benchmarking. The axon client has no NTFF profile hook and per-dispatch
# overhead is ~0.5-1 ms (noisy), so we time via in-NEFF replication: build
# the same kernel with the body replicated R times inside one NEFF, and use
# slope (t(R) - t(1)) / (R - 1) with min-of-N dispatches.
# ---------------------------------------------------------------------------


def _make_callable(nc, in_maps):
    import jax
    from jax.sharding import Mesh, NamedSharding, PartitionSpec
    from jax.experimental.shard_map import shard_map
    from concourse import bass2jax, mybir as _mb

    bass2jax.install_neuronx_cc_hook()

    in_names, out_names, out_avals, zero_outs = [], [], [], []
    partition_name = (
        nc.partition_id_tensor.name if nc.partition_id_tensor else None
    )
    for alloc in nc.m.functions[0].allocations:
        if not isinstance(alloc, _mb.MemoryLocationSet):
            continue
        name = alloc.memorylocations[0].name
        if alloc.kind == "ExternalInput":
            if name != partition_name:
                in_names.append(name)
        elif alloc.kind == "ExternalOutput":
            shape = tuple(alloc.tensor_shape)
            dtype = _mb.dt.np(alloc.dtype)
            out_names.append(name)
            out_avals.append(jax.core.ShapedArray(shape, dtype))
            zero_outs.append(np.zeros(shape, dtype))
    n_params = len(in_names)
    all_in_names = list(in_names) + list(out_names)
    if partition_name is not None:
        all_in_names.append(partition_name)

    def _body(*args):
        operands = list(args)
        if partition_name is not None:
            operands.append(bass2jax.partition_id_tensor())
        outs = bass2jax._bass_exec_p.bind(
            *operands,
            out_avals=tuple(out_avals),
            in_names=tuple(all_in_names),
            out_names=tuple(out_names),
            lowering_input_output_aliases=(),
            sim_require_finite=True,
            sim_require_nnan=True,
            nc=nc,
        )
        return tuple(outs)

    devices = jax.devices()[:N_CORES]
    mesh = Mesh(np.asarray(devices), ("core",))
    in_specs = (PartitionSpec("core"),) * (n_params + len(out_names))
    out_specs = (PartitionSpec("core"),) * len(out_names)
    fn = jax.jit(
        shard_map(
            _body, mesh=mesh, in_specs=in_specs, out_specs=out_specs,
            check_rep=False,
        ),
        keep_unused=True,
    )
    concat_in = [
        np.concatenate([np.asarray(in_maps[c][k]) for c in range(N_CORES)], axis=0)
        for k in in_names
    ]
    concat_zero = [
        np.concatenate([z for _ in range(N_CORES)], axis=0) for z in zero_outs
    ]
    sh = NamedSharding(mesh, PartitionSpec("core"))
    dev_args = [jax.device_put(a, sh) for a in concat_in + concat_zero]
    return fn, dev_args


def bench_programs(nc1, ncR, in_maps, big_reps, rounds=9, b_small=4, b_large=28):
    """Median-of-rounds estimate of per-rep HW time between a 1-rep and an
    R-rep NEFF.  Each round measures both marginals back-to-back
    (interleaved) so slow drift in the per-dispatch axon overhead cancels.
    """
    import time as _time
    import jax

    fn1, args1 = _make_callable(nc1, in_maps)
    fnR, argsR = _make_callable(ncR, in_maps)

    for fn, args in ((fn1, args1), (fnR, argsR)):
        for _ in range(3):  # warm up compile + execution
            jax.block_until_ready(fn(*args))

    def batch_time(fn, args, k):
        t0 = _time.perf_counter()
        r = None
        for _ in range(k):
            r = fn(*args)
        jax.block_until_ready(r)
        return _time.perf_counter() - t0, r

    def marginal(fn, args):
        ta, _ = batch_time(fn, args, b_small)
        tb, res = batch_time(fn, args, b_large)
        return (tb - ta) / (b_large - b_small), res

    estimates = []
    res1 = None
    for _ in range(rounds):
        m1, res1 = marginal(fn1, args1)
        mR, _ = marginal(fnR, argsR)
        estimates.append((mR - m1) / (big_reps - 1))
    estimates.sort()
    # Per-dispatch axon overhead noise is one-sided (load spikes only add
    # time), so a lower quantile estimates the true HW time better than
    # the median.
    est = estimates[len(estimates) // 4]
    return est * 1e9, np.asarray(estimates) * 1e9, res1


def bench(x, y, W, b, mask, big_reps=33, rounds=9):
    """Returns (per_iter_ns, out) via in-NEFF replication, median estimate."""
    in_maps = make_in_maps(x, y, W, b, mask)
    med, ests, res1 = bench_programs(
        _get_nc(1), _get_nc(big_reps), in_maps, big_reps, rounds=rounds
    )
    print(
        "bench estimates (ns):",
        " ".join(f"{e:.0f}" for e in ests),
    )
    out = np.asarray(res1[0])
    return med, out

